# revision 52
# baseline (speedup 1.0000x reference)
"""DoRA Linear on 8 Trainium2 NeuronCores (Bass/Tile), fp16 + fp8-DoubleRow.

Reference computation (all fp32):
    new_v   = base_weight + SCALE * dora_B @ dora_A          [OUT, IN]
    scale_o = weight_m / ||new_v||_row                        [OUT]
    out     = x @ (scale_o[:, None] * new_v)^T + base_bias    [B, S, OUT]

Sharding: column-parallel over OUT across 8 cores (OUT/8 = 512 each).
base_weight, dora_B, weight_m, base_bias sharded; x, dora_A replicated.

The main matmul runs in mixed precision to stay under the 2e-2 rel-err
budget while using fp8 DoubleRow (2x PE throughput) where possible:
  - k-chunks 0..23 (3072 of 4096 contraction): fp16 weights and x.
  - k-chunks 24..31: e4m3 fp8, perf_mode=DoubleRow, two 128-k chunks
    per matmul.  Measured error of this 24/8 split: 1.88e-2 (9 fp8
    chunks would be 1.99e-2 - too close to the gate).
Everything is pre-scaled so both parts accumulate in one PSUM group:
weights carry x1024, x carries x16; the /16384 rides in scale_o.

Per-core device program:
  1. Build W'^T = (1024 W + 2048 B@A)^T chunk-by-chunk: PE matmul
     A^T@(2048 B^T) -> PSUM, DVE adds the fp16 1024*W^T chunk, writing
     fp16 wr16 (k<24) or fp8 wr8 pairs (k>=24).
  2. Row norms of the QUANTIZED scaled weights: ACT computes
     sq8 = Square(wr * 2^-5) into e4m3 pairs, PE accumulates
     ones8^T @ sq8 with DoubleRow norm matmuls (one PSUM group, 16 MMs).
     norm matmuls keep M = 128 output partitions (smaller M compiles
     but the runtime refuses the NEFF).
  3. scale_col = (wm/512) / sqrt(nr): PE transpose lands the norms on
     o-partitions, ACT sqrt, DVE reciprocal/mul.  (nr = norm2 * 2^-10.)
  4. Main matmul outT[o, m] = sum_k wr[k, o] * xs[k, m]: 24 fp16 MMs +
     4 fp8 DoubleRow MMs per PSUM group; eviction fuses *scale_o +
     bias_o in one DVE tensor_scalar, output stored fp16 (host upcasts).
Scheduling: ~3.5us of dummy matmuls release the HAM PE clock-gate
(1.2 -> 2.4 GHz) before the real stream; m-chunk 0 matmuls trail the
weight build by one k-pair and norm matmuls by two, so the PE never
waits on the DVE-add -> ACT-square chain.  The whole fp8 x panel
(64KB/partition) is SBUF-resident, streamed in per-m-chunk slices with
+2 lookahead.  All DMAs ride one (sync) queue, ordered critical-first
- a parallel-queue bulk load starves the shared HW DMA engines, and
SWDGE (gpsimd) stores cost a ~7us queue-teardown DRAIN in the tail.
The last m-chunk runs oc-outer so the tail is one eviction + one
small fp16 store.
Host: layout transposes + dtype casts in numpy, final gather/transpose.
"""

import numpy as np
import ml_dtypes

import concourse.mybir as mybir
import concourse.tile as tile
from concourse import bacc
from concourse.bass_utils import run_bass_kernel_spmd
from concourse.masks import make_identity

OUT, IN, RANK = 4096, 4096, 16
SCALE = 2.0
NCORES = 8
OSH = OUT // NCORES          # 512 out features per core
P = 128
KO = IN // P                 # 32 k-chunks
KO16 = 24                    # fp16 k-chunks
KP8 = (KO - KO16) // 2       # 4 fp8 k-pairs (DoubleRow)
KQ = 4                       # k-quarters of 8 chunks (3 fp16 + 1 fp8)
KO_Q = 8
M = 4 * 2048                 # 8192 tokens
MCH = 512                    # tokens per x tile
NM = M // MCH                # 16 m-chunks
OC = OSH // P                # 4 o-chunks of 128
SW = 1024.0                  # weight pre-scale (host)
SX = 16.0                    # x pre-scale (host)
SQS = 2.0 ** -5              # ACT scale for squares: sq = wr^2 * 2^-10

F32 = mybir.dt.float32
F16 = mybir.dt.float16
F8 = mybir.dt.float8e4
DR = mybir.MatmulPerfMode.DoubleRow
ADD = mybir.AluOpType.add
MULT = mybir.AluOpType.mult


def _build():
    nc = bacc.Bacc(None, target_bir_lowering=False)
    xT16 = nc.dram_tensor("xT16", [P, KO16, M], F16, kind="ExternalInput")
    xT8 = nc.dram_tensor("xT8", [P, KP8, 2, M], F8, kind="ExternalInput")
    wT = nc.dram_tensor("wT", [P, KO, OSH], F16, kind="ExternalInput")
    # A chunks for even k at partitions 0-15, odd k at 32-47; B^T
    # duplicated to match: the two B@A matmuls of a k-pair run
    # CONCURRENTLY in disjoint 32-row groups of the PE array
    aT = nc.dram_tensor("aT", [2 * 32, KO // 2, P], F16,
                        kind="ExternalInput")
    bT = nc.dram_tensor("bT", [2 * 32, OSH], F32, kind="ExternalInput")
    wm = nc.dram_tensor("wm", [P, OC], F32, kind="ExternalInput")
    bc = nc.dram_tensor("bc", [P, OC], F32, kind="ExternalInput")
    outT = nc.dram_tensor("outT", [OSH, M], F16, kind="ExternalOutput")
    outT_v = outT.ap().rearrange("(oc p) m -> oc p m", p=P)

    with tile.TileContext(nc) as tc:
        with (
            tc.tile_pool(name="wr", bufs=1) as wrpool,
            tc.tile_pool(name="const", bufs=1) as cpool,
            tc.tile_pool(name="wv", bufs=2) as wvpool,
            tc.tile_pool(name="sq", bufs=3) as sqpool,
            tc.tile_pool(name="xs", bufs=6) as xpool,
            tc.tile_pool(name="os", bufs=4) as opool,
            tc.tile_pool(name="ps_mm", bufs=8, space="PSUM") as ps_mm,
        ):
            # ---- critical-first loads (single sync DMA queue) ----
            bt_f = cpool.tile([2 * 32, OSH], F32)
            nc.sync.dma_start(bt_f[:], bT.ap())
            at_s = cpool.tile([2 * 32, KO // 2, P], F16)
            nc.sync.dma_start(at_s[:], aT.ap())
            # the whole fp8 x panel is only 64KB/partition - keep it
            # SBUF-resident so no DoubleRow matmul ever waits on a DMA
            # gate.  Loads ride the sync queue AFTER the prep-critical
            # pushes (a parallel-queue load hogs the shared HW DMA
            # engines and starves the prep stream for ~24us).
            xr8 = cpool.tile([P, KP8, 2, M], F8)
            ones_f = cpool.tile([P, 2 * P], F32)
            nc.any.memset(ones_f[:], 1.0)
            # DVE order matters: ones8 first (no DMA dep) so warm-up
            # matmuls can issue while bt_f is still in flight
            ones8 = cpool.tile([P, 2, P], F8)
            nc.vector.tensor_copy(ones8[:], ones_f[:])
            bt2 = cpool.tile([2 * 32, OSH], F16)
            nc.vector.tensor_scalar_mul(bt2[:], bt_f[:], SCALE * SW)
            wm_col = cpool.tile([P, OC], F32)
            bias_col = cpool.tile([P, OC], F32)

            # ---- weight prep + m-chunk 0, interleaved per k-pair:
            # wr[:, ko] = 1024*W^T chunk + (2048 B A)^T chunk ----
            wr16 = wrpool.tile([P, KO16, OSH], F16)
            wr8 = wrpool.tile([P, KP8, 2, OSH], F8)
            nr = ps_mm.tile([P, OSH], F32, name="mm")
            # HAM warm-up: ~3.5us of dummy matmuls on ones8 so the PE
            # clock-gate releases (1.2 -> 2.4 GHz) before the real
            # instruction stream begins
            for _ in range(33):
                nc.tensor.matmul(nr[:, 0:P], ones8[:, 0], ones8[:, 0],
                                 start=True, stop=True)
            pss0 = [ps_mm.tile([P, MCH], F32, name="mm") for _ in range(OC)]
            xq_tiles = {}

            def emit_mc0(pair, half=None):
                if pair < 0:
                    return
                if pair < KO16 // 2:
                    kq = pair // (KO_Q // 2)
                    ts = (0, 1) if half is None else (half,)
                    for t in ts:
                        ko = pair * 2 + t
                        k8 = ko - kq * KO_Q
                        if kq == 0:
                            xtf_, xtr_ = xq_tiles[0]
                            xsrc = xtf_[:, k8] if k8 < 2 else xtr_[:, k8 - 2]
                        else:
                            xsrc = xq_tiles[kq][:, k8]
                        for oc in range(OC):
                            nc.tensor.matmul(
                                pss0[oc][:],
                                wr16[:, ko, oc * P:(oc + 1) * P],
                                xsrc,
                                start=(ko == 0), stop=False)
                else:
                    if half == 1:
                        return
                    jp8 = pair - KO16 // 2
                    for oc in range(OC):
                        nc.tensor.matmul(
                            pss0[oc][:],
                            wr8[:, jp8, :, oc * P:(oc + 1) * P],
                            xr8[:, jp8, :, 0:MCH],
                            start=False, stop=(jp8 == KP8 - 1),
                            perf_mode=DR)

            pend_sq = []
            for kq in range(KQ):
                # fine-grained tiles around the critical path: DMA
                # completion gates are whole-tile, so the first chunks'
                # weights/x get their own small tiles
                wva = wvpool.tile([P, 4, OSH], F16, name="wva")
                wvb = wvpool.tile([P, 4, OSH], F16, name="wvb")
                q0 = kq * KO_Q
                nc.sync.dma_start(wva[:], wT.ap()[:, q0:q0 + 4])
                if kq == 0:
                    # single-use tiles -> cpool (bufs=1), keeps the
                    # xpool ring small enough for SBUF
                    xtf = cpool.tile([P, 2, MCH], F16)
                    nc.sync.dma_start(xtf[:], xT16.ap()[:, 0:2, 0:MCH])
                    xt0 = cpool.tile([P, 6, MCH], F16)
                    nc.sync.dma_start(xt0[:], xT16.ap()[:, 2:KO_Q, 0:MCH])
                    xq_tiles[0] = (xtf, xt0)
                elif kq < 3:
                    xt0 = xpool.tile([P, KO_Q, MCH], F16, name="xt")
                    nc.sync.dma_start(
                        xt0[:], xT16.ap()[:, q0:q0 + KO_Q, 0:MCH])
                    xq_tiles[kq] = xt0
                nc.sync.dma_start(wvb[:], wT.ap()[:, q0 + 4:q0 + KO_Q])
                if kq == 1:
                    # m-chunk 0 slice of the fp8 x panel (needed ~45us)
                    nc.sync.dma_start(xr8[:, :, :, 0:MCH],
                                      xT8.ap()[:, :, :, 0:MCH])
                if kq == 3:
                    # small, needed only at scale_col time
                    nc.sync.dma_start(wm_col[:], wm.ap())
                    nc.sync.dma_start(bias_col[:], bc.ap())

                for jp in range(KO_Q // 2):
                    sq8 = sqpool.tile([P, 2, OSH], F8, name="sq8")
                    pair = kq * (KO_Q // 2) + jp
                    bas = [ps_mm.tile([P, OSH], F32, name="mm")
                           for _ in range(2)]
                    for t in range(2):
                        nc.tensor.matmul(
                            bas[t][:], at_s[32 * t:32 * t + RANK, pair],
                            bt2[32 * t:32 * t + RANK],
                            start=True, stop=True)
                    for t in range(2):
                        k8 = 2 * jp + t
                        ko = kq * KO_Q + k8
                        if ko < KO16:
                            wdst = wr16[:, ko]
                        else:
                            wdst = wr8[:, (ko - KO16) // 2, (ko - KO16) % 2]
                        wsrc = wva[:, k8] if k8 < 4 else wvb[:, k8 - 4]
                        nc.vector.tensor_tensor(wdst, wsrc, bas[t][:], ADD)
                        nc.scalar.activation(
                            sq8[:, t], wdst,
                            mybir.ActivationFunctionType.Square, scale=SQS)
                    # m-chunk 0 matmuls one pair behind the weight
                    # build (never waiting on the DVE adds); norm
                    # matmuls trail TWO pairs so the ba->add->square
                    # chain never paces the PE
                    emit_mc0(pair - 1, half=0)
                    if len(pend_sq) == 2:
                        nc.tensor.matmul(
                            nr[:], ones8[:], pend_sq.pop(0)[:],
                            start=(pair == 2), stop=False, perf_mode=DR)
                    pend_sq.append(sq8)
                    emit_mc0(pair - 1, half=1)
            nc.tensor.matmul(
                nr[:], ones8[:], pend_sq[0][:],
                start=False, stop=False, perf_mode=DR)
            emit_mc0(KO // 2 - 1)
            nc.tensor.matmul(
                nr[:], ones8[:], pend_sq[1][:],
                start=False, stop=True, perf_mode=DR)
            # fp8 x panel slices for m-chunks 1+2 (the rest prefetch
            # inside the main loop with +2 m-chunk lookahead)
            for j in (1, 2):
                nc.sync.dma_start(xr8[:, :, :, j * MCH:(j + 1) * MCH],
                                  xT8.ap()[:, :, :, j * MCH:(j + 1) * MCH])

            # ---- scale_col = (wm/512) / sqrt(nr): every row of nr
            # holds the same 512 norms; PE-transpose 128-wide chunks to
            # land them on o-partitions (no DRAM bounce - that path
            # yields a NEFF the runtime refuses to load) ----
            ident = cpool.tile([P, P], F32)
            make_identity(nc, ident)
            sqc = cpool.tile([P, OC], F32)
            # one copy releases the nr PSUM bank for m-chunk 1's groups
            nr_sb = sqpool.tile([P, OSH], F32, name="nrb")
            nc.vector.tensor_copy(nr_sb[:], nr[:])
            for oc in range(OC):
                pt = ps_mm.tile([P, P], F32, name="mm")
                nc.tensor.transpose(
                    pt[:], nr_sb[:, oc * P:(oc + 1) * P], ident[:])
                nc.scalar.activation(
                    sqc[:, oc:oc + 1], pt[:, 0:1],
                    mybir.ActivationFunctionType.Sqrt)
            rcp = cpool.tile([P, OC], F32)
            nc.vector.reciprocal(rcp[:], sqc[:])
            scale_col = cpool.tile([P, OC], F32)
            nc.vector.tensor_tensor(scale_col[:], wm_col[:], rcp[:], MULT)

            # ---- m-chunk 0 eviction ----
            for oc in range(OC):
                ot0 = opool.tile([P, MCH], F16, name="ot")
                nc.vector.tensor_scalar(
                    ot0[:], pss0[oc][:],
                    scale_col[:, oc:oc + 1], bias_col[:, oc:oc + 1],
                    MULT, ADD)
                nc.sync.dma_start(outT_v[oc, :, 0:MCH], ot0[:])

            # ---- main matmul: outT[o, m] accumulated over k ----
            for mc in range(1, NM):
                pss = [ps_mm.tile([P, MCH], F32, name="mm")
                       for _ in range(OC)]
                if mc + 2 < NM:
                    j = mc + 2
                    nc.sync.dma_start(
                        xr8[:, :, :, j * MCH:(j + 1) * MCH],
                        xT8.ap()[:, :, :, j * MCH:(j + 1) * MCH])
                xts = []
                for kq in range(3):
                    xt = xpool.tile([P, KO_Q, MCH], F16, name="xt")
                    nc.sync.dma_start(
                        xt[:],
                        xT16.ap()[:, kq * KO_Q:(kq + 1) * KO_Q,
                                  mc * MCH:(mc + 1) * MCH])
                    xts.append(xt)

                if mc < NM - 1:
                    for kq in range(3):
                        for oc in range(OC):
                            for k8 in range(KO_Q):
                                nc.tensor.matmul(
                                    pss[oc][:],
                                    wr16[:, kq * KO_Q + k8,
                                         oc * P:(oc + 1) * P],
                                    xts[kq][:, k8],
                                    start=(kq == 0 and k8 == 0), stop=False)
                    for oc in range(OC):
                        for kp in range(KP8):
                            nc.tensor.matmul(
                                pss[oc][:],
                                wr8[:, kp, :, oc * P:(oc + 1) * P],
                                xr8[:, kp, :, mc * MCH:(mc + 1) * MCH],
                                start=False, stop=(kp == KP8 - 1),
                                perf_mode=DR)
                    for oc in range(OC):
                        ot = opool.tile([P, MCH], F16)
                        nc.vector.tensor_scalar(
                            ot[:], pss[oc][:],
                            scale_col[:, oc:oc + 1], bias_col[:, oc:oc + 1],
                            MULT, ADD)
                        nc.sync.dma_start(
                            outT_v[oc, :, mc * MCH:(mc + 1) * MCH], ot[:])
                else:
                    # last m-chunk: oc-outer so each oc's accumulation
                    # closes early and eviction/store overlap the
                    # remaining groups - shortens the kernel tail
                    for oc in range(OC):
                        for kq in range(3):
                            for k8 in range(KO_Q):
                                nc.tensor.matmul(
                                    pss[oc][:],
                                    wr16[:, kq * KO_Q + k8,
                                         oc * P:(oc + 1) * P],
                                    xts[kq][:, k8],
                                    start=(kq == 0 and k8 == 0), stop=False)
                        for kp in range(KP8):
                            nc.tensor.matmul(
                                pss[oc][:],
                                wr8[:, kp, :, oc * P:(oc + 1) * P],
                                xr8[:, kp, :, mc * MCH:(mc + 1) * MCH],
                                start=False, stop=(kp == KP8 - 1),
                                perf_mode=DR)
                        ot = opool.tile([P, MCH], F16)
                        nc.vector.tensor_scalar(
                            ot[:], pss[oc][:],
                            scale_col[:, oc:oc + 1], bias_col[:, oc:oc + 1],
                            MULT, ADD)
                        nc.sync.dma_start(
                            outT_v[oc, :, mc * MCH:(mc + 1) * MCH], ot[:])
    nc.compile()
    return nc


def kernel(x, base_weight, base_bias, weight_m, dora_A, dora_B):
    x = np.asarray(x, dtype=np.float32)
    base_weight = np.asarray(base_weight, dtype=np.float32)
    base_bias = np.asarray(base_bias, dtype=np.float32)
    weight_m = np.asarray(weight_m, dtype=np.float32)
    dora_A = np.asarray(dora_A, dtype=np.float32)
    dora_B = np.asarray(dora_B, dtype=np.float32)

    B, S, _ = x.shape
    assert B * S == M and x.shape[2] == IN

    # x layouts (shared across all cores), pre-scaled by 16:
    #   xT16[p, ko, m] = 16*x[m, ko*128+p]          fp16, ko < 24
    #   xT8[p, kp, t, m] = q8(16*x[m, (24+2kp+t)*128+p])  e4m3
    xs = (x.reshape(M, KO, P) * SX)
    xT16 = np.ascontiguousarray(
        xs[:, :KO16].transpose(2, 1, 0)).astype(np.float16)
    x8part = xs[:, KO16:].reshape(M, KP8, 2, P).transpose(3, 1, 2, 0)
    xT8 = np.clip(np.ascontiguousarray(x8part), -240, 240).astype(
        ml_dtypes.float8_e4m3)
    # A chunks: even k-chunks at partitions 0-15, odd at 32-47
    aT2 = np.zeros((64, KO // 2, P), dtype=np.float16)
    a3 = dora_A.reshape(RANK, KO // 2, 2, P)
    aT2[0:RANK] = a3[:, :, 0]
    aT2[32:32 + RANK] = a3[:, :, 1]

    in_maps = []
    for c in range(NCORES):
        sl = slice(c * OSH, (c + 1) * OSH)
        w_c = base_weight[sl] * SW                              # [OSH, IN]
        wT_c = np.ascontiguousarray(
            w_c.reshape(OSH, KO, P).transpose(2, 1, 0)).astype(np.float16)
        bT_c = np.zeros((64, OSH), dtype=np.float32)
        bT_c[0:RANK] = dora_B[sl].T
        bT_c[32:32 + RANK] = dora_B[sl].T
        wm_c = np.ascontiguousarray(
            (weight_m[sl] / (SX * 32.0)).reshape(OC, P).T)
        bc_c = np.ascontiguousarray(base_bias[sl].reshape(OC, P).T)
        in_maps.append({
            "xT16": xT16,
            "xT8": xT8,
            "wT": wT_c,
            "aT": aT2,
            "bT": bT_c,
            "wm": wm_c,
            "bc": bc_c,
        })

    nc = _build()
    res = run_bass_kernel_spmd(nc, in_maps, core_ids=list(range(NCORES)))

    full = np.empty((OUT, M), dtype=np.float32)
    for c in range(NCORES):
        full[c * OSH:(c + 1) * OSH] = res.results[c]["outT"].astype(
            np.float32)
    return np.ascontiguousarray(full.T).reshape(B, S, OUT)


# revision 53
# speedup vs baseline: 1.0078x; 1.0078x over previous
"""DoRA Linear on 8 Trainium2 NeuronCores (Bass/Tile), fp16 + fp8-DoubleRow.

Reference computation (all fp32):
    new_v   = base_weight + SCALE * dora_B @ dora_A          [OUT, IN]
    scale_o = weight_m / ||new_v||_row                        [OUT]
    out     = x @ (scale_o[:, None] * new_v)^T + base_bias    [B, S, OUT]

Sharding: column-parallel over OUT across 8 cores (OUT/8 = 512 each).
base_weight, dora_B, weight_m, base_bias sharded; x, dora_A replicated.

The main matmul runs in mixed precision to stay under the 2e-2 rel-err
budget while using fp8 DoubleRow (2x PE throughput) where possible:
  - k-chunks 0..23 (3072 of 4096 contraction): fp16 weights and x.
  - k-chunks 24..31: e4m3 fp8, perf_mode=DoubleRow, two 128-k chunks
    per matmul.  Measured error of this 24/8 split: 1.88e-2 (9 fp8
    chunks would be 1.99e-2 - too close to the gate).
Everything is pre-scaled so both parts accumulate in one PSUM group:
weights carry x1024, x carries x16; the /16384 rides in scale_o.

Per-core device program:
  1. Build W'^T = (1024 W + 2048 B@A)^T chunk-by-chunk: PE matmul
     A^T@(2048 B^T) -> PSUM, DVE adds the fp16 1024*W^T chunk, writing
     fp16 wr16 (k<24) or fp8 wr8 pairs (k>=24).
  2. Row norms of the QUANTIZED scaled weights: ACT computes
     sq8 = Square(wr * 2^-5) into e4m3 pairs, PE accumulates
     ones8^T @ sq8 with DoubleRow norm matmuls (one PSUM group, 16 MMs).
     norm matmuls keep M = 128 output partitions (smaller M compiles
     but the runtime refuses the NEFF).
  3. scale_col = (wm/512) / sqrt(nr): PE transpose lands the norms on
     o-partitions, ACT sqrt, DVE reciprocal/mul.  (nr = norm2 * 2^-10.)
  4. Main matmul outT[o, m] = sum_k wr[k, o] * xs[k, m]: 24 fp16 MMs +
     4 fp8 DoubleRow MMs per PSUM group; eviction fuses *scale_o +
     bias_o in one DVE tensor_scalar, output stored fp16 (host upcasts).
Scheduling: ~3.5us of dummy matmuls release the HAM PE clock-gate
(1.2 -> 2.4 GHz) before the real stream; m-chunk 0 matmuls trail the
weight build by one k-pair and norm matmuls by two, so the PE never
waits on the DVE-add -> ACT-square chain.  The whole fp8 x panel
(64KB/partition) is SBUF-resident, streamed in per-m-chunk slices with
+2 lookahead.  All DMAs ride one (sync) queue, ordered critical-first
- a parallel-queue bulk load starves the shared HW DMA engines, and
SWDGE (gpsimd) stores cost a ~7us queue-teardown DRAIN in the tail.
The last m-chunk runs oc-outer so the tail is one eviction + one
small fp16 store.
Host: layout transposes + dtype casts in numpy, final gather/transpose.
"""

import numpy as np
import ml_dtypes

import concourse.mybir as mybir
import concourse.tile as tile
from concourse import bacc
from concourse.bass_utils import run_bass_kernel_spmd
from concourse.masks import make_identity

OUT, IN, RANK = 4096, 4096, 16
SCALE = 2.0
NCORES = 8
OSH = OUT // NCORES          # 512 out features per core
P = 128
KO = IN // P                 # 32 k-chunks
KO16 = 24                    # fp16 k-chunks
KP8 = (KO - KO16) // 2       # 4 fp8 k-pairs (DoubleRow)
KQ = 4                       # k-quarters of 8 chunks (3 fp16 + 1 fp8)
KO_Q = 8
M = 4 * 2048                 # 8192 tokens
MCH = 512                    # tokens per x tile
NM = M // MCH                # 16 m-chunks
OC = OSH // P                # 4 o-chunks of 128
SW = 1024.0                  # weight pre-scale (host)
SX = 16.0                    # x pre-scale (host)
SQS = 2.0 ** -5              # ACT scale for squares: sq = wr^2 * 2^-10

F32 = mybir.dt.float32
F16 = mybir.dt.float16
F8 = mybir.dt.float8e4
DR = mybir.MatmulPerfMode.DoubleRow
ADD = mybir.AluOpType.add
MULT = mybir.AluOpType.mult


def _build():
    nc = bacc.Bacc(None, target_bir_lowering=False)
    xT16 = nc.dram_tensor("xT16", [P, KO16, M], F16, kind="ExternalInput")
    xT8 = nc.dram_tensor("xT8", [P, KP8, 2, M], F8, kind="ExternalInput")
    wT = nc.dram_tensor("wT", [P, KO, OSH], F16, kind="ExternalInput")
    # A chunks for even k at partitions 0-15, odd k at 32-47; B^T
    # duplicated to match: the two B@A matmuls of a k-pair run
    # CONCURRENTLY in disjoint 32-row groups of the PE array
    aT = nc.dram_tensor("aT", [2 * 32, KO // 2, P], F16,
                        kind="ExternalInput")
    bT = nc.dram_tensor("bT", [2 * 32, OSH], F32, kind="ExternalInput")
    wm = nc.dram_tensor("wm", [P, OC], F32, kind="ExternalInput")
    bc = nc.dram_tensor("bc", [P, OC], F32, kind="ExternalInput")
    outT = nc.dram_tensor("outT", [OSH, M], F16, kind="ExternalOutput")
    outT_v = outT.ap().rearrange("(oc p) m -> oc p m", p=P)

    with tile.TileContext(nc) as tc:
        with (
            tc.tile_pool(name="wr", bufs=1) as wrpool,
            tc.tile_pool(name="const", bufs=1) as cpool,
            tc.tile_pool(name="wv", bufs=2) as wvpool,
            tc.tile_pool(name="sq", bufs=3) as sqpool,
            tc.tile_pool(name="xs", bufs=6) as xpool,
            tc.tile_pool(name="os", bufs=4) as opool,
            tc.tile_pool(name="ps_mm", bufs=8, space="PSUM") as ps_mm,
        ):
            # ---- critical-first loads (single sync DMA queue) ----
            bt_f = cpool.tile([2 * 32, OSH], F32)
            nc.sync.dma_start(bt_f[:], bT.ap())
            at_s = cpool.tile([2 * 32, KO // 2, P], F16)
            nc.sync.dma_start(at_s[:], aT.ap())
            # the whole fp8 x panel is only 64KB/partition - keep it
            # SBUF-resident so no DoubleRow matmul ever waits on a DMA
            # gate.  Loads ride the sync queue AFTER the prep-critical
            # pushes (a parallel-queue load hogs the shared HW DMA
            # engines and starves the prep stream for ~24us).
            xr8 = cpool.tile([P, KP8, 2, M], F8)
            ones_f = cpool.tile([P, 2 * P], F32)
            nc.any.memset(ones_f[:], 1.0)
            # DVE order matters: ones8 first (no DMA dep) so warm-up
            # matmuls can issue while bt_f is still in flight
            ones8 = cpool.tile([P, 2, P], F8)
            nc.vector.tensor_copy(ones8[:], ones_f[:])
            bt2 = cpool.tile([2 * 32, OSH], F16)
            nc.vector.tensor_scalar_mul(bt2[:], bt_f[:], SCALE * SW)
            wm_col = cpool.tile([P, OC], F32)
            bias_col = cpool.tile([P, OC], F32)

            # ---- weight prep + m-chunk 0, interleaved per k-pair:
            # wr[:, ko] = 1024*W^T chunk + (2048 B A)^T chunk ----
            wr16 = wrpool.tile([P, KO16, OSH], F16)
            wr8 = wrpool.tile([P, KP8, 2, OSH], F8)
            nr = ps_mm.tile([P, OSH], F32, name="mm")
            # HAM warm-up: ~3.5us of dummy matmuls on ones8 so the PE
            # clock-gate releases (1.2 -> 2.4 GHz) before the real
            # instruction stream begins
            for _ in range(33):
                nc.tensor.matmul(nr[:, 0:P], ones8[:, 0], ones8[:, 0],
                                 start=True, stop=True)
            pss0 = [ps_mm.tile([P, MCH], F32, name="mm") for _ in range(OC)]
            xq_tiles = {}

            def emit_mc0(pair, half=None):
                if pair < 0:
                    return
                if pair < KO16 // 2:
                    kq = pair // (KO_Q // 2)
                    ts = (0, 1) if half is None else (half,)
                    for t in ts:
                        ko = pair * 2 + t
                        k8 = ko - kq * KO_Q
                        if kq == 0:
                            xtf_, xtr_ = xq_tiles[0]
                            xsrc = xtf_[:, k8] if k8 < 2 else xtr_[:, k8 - 2]
                        else:
                            xsrc = xq_tiles[kq][:, k8]
                        for oc in range(OC):
                            nc.tensor.matmul(
                                pss0[oc][:],
                                wr16[:, ko, oc * P:(oc + 1) * P],
                                xsrc,
                                start=(ko == 0), stop=False)
                else:
                    if half == 1:
                        return
                    jp8 = pair - KO16 // 2
                    for oc in range(OC):
                        nc.tensor.matmul(
                            pss0[oc][:],
                            wr8[:, jp8, :, oc * P:(oc + 1) * P],
                            xr8[:, jp8, :, 0:MCH],
                            start=False, stop=(jp8 == KP8 - 1),
                            perf_mode=DR)

            pend_sq = []
            for kq in range(KQ):
                # fine-grained tiles around the critical path: DMA
                # completion gates are whole-tile, so the first chunks'
                # weights/x get their own small tiles
                wva = wvpool.tile([P, 4, OSH], F16, name="wva")
                wvb = wvpool.tile([P, 4, OSH], F16, name="wvb")
                q0 = kq * KO_Q
                nc.sync.dma_start(wva[:], wT.ap()[:, q0:q0 + 4])
                if kq == 0:
                    # single-use tiles -> cpool (bufs=1), keeps the
                    # xpool ring small enough for SBUF
                    xtf = cpool.tile([P, 2, MCH], F16)
                    nc.sync.dma_start(xtf[:], xT16.ap()[:, 0:2, 0:MCH])
                    xt0 = cpool.tile([P, 6, MCH], F16)
                    nc.sync.dma_start(xt0[:], xT16.ap()[:, 2:KO_Q, 0:MCH])
                    xq_tiles[0] = (xtf, xt0)
                elif kq < 3:
                    xt0 = xpool.tile([P, KO_Q, MCH], F16, name="xt")
                    nc.sync.dma_start(
                        xt0[:], xT16.ap()[:, q0:q0 + KO_Q, 0:MCH])
                    xq_tiles[kq] = xt0
                nc.sync.dma_start(wvb[:], wT.ap()[:, q0 + 4:q0 + KO_Q])
                if kq == 1:
                    # m-chunk 0 slice of the fp8 x panel (needed ~45us)
                    nc.sync.dma_start(xr8[:, :, :, 0:MCH],
                                      xT8.ap()[:, :, :, 0:MCH])
                if kq == 3:
                    # small, needed only at scale_col time
                    nc.sync.dma_start(wm_col[:], wm.ap())
                    nc.sync.dma_start(bias_col[:], bc.ap())

                for jp in range(KO_Q // 2):
                    sq8 = sqpool.tile([P, 2, OSH], F8, name="sq8")
                    pair = kq * (KO_Q // 2) + jp
                    bas = [ps_mm.tile([P, OSH], F32, name="mm")
                           for _ in range(2)]
                    for t in range(2):
                        nc.tensor.matmul(
                            bas[t][:], at_s[32 * t:32 * t + RANK, pair],
                            bt2[32 * t:32 * t + RANK],
                            start=True, stop=True)
                    for t in range(2):
                        k8 = 2 * jp + t
                        ko = kq * KO_Q + k8
                        if ko < KO16:
                            wdst = wr16[:, ko]
                        else:
                            wdst = wr8[:, (ko - KO16) // 2, (ko - KO16) % 2]
                        wsrc = wva[:, k8] if k8 < 4 else wvb[:, k8 - 4]
                        nc.vector.tensor_tensor(wdst, wsrc, bas[t][:], ADD)
                        nc.scalar.activation(
                            sq8[:, t], wdst,
                            mybir.ActivationFunctionType.Square, scale=SQS)
                    # m-chunk 0 matmuls one pair behind the weight
                    # build (never waiting on the DVE adds); norm
                    # matmuls trail TWO pairs so the ba->add->square
                    # chain never paces the PE
                    emit_mc0(pair - 1, half=0)
                    if len(pend_sq) == 2:
                        nc.tensor.matmul(
                            nr[:], ones8[:], pend_sq.pop(0)[:],
                            start=(pair == 2), stop=False, perf_mode=DR)
                    pend_sq.append(sq8)
                    emit_mc0(pair - 1, half=1)
            nc.tensor.matmul(
                nr[:], ones8[:], pend_sq[0][:],
                start=False, stop=False, perf_mode=DR)
            emit_mc0(KO // 2 - 1)
            nc.tensor.matmul(
                nr[:], ones8[:], pend_sq[1][:],
                start=False, stop=True, perf_mode=DR)
            # fp8 x panel slices for m-chunks 1+2 (the rest prefetch
            # inside the main loop with +2 m-chunk lookahead)
            for j in (1, 2):
                nc.sync.dma_start(xr8[:, :, :, j * MCH:(j + 1) * MCH],
                                  xT8.ap()[:, :, :, j * MCH:(j + 1) * MCH])

            # ---- scale_col = (wm/512) / sqrt(nr): every row of nr
            # holds the same 512 norms; PE-transpose 128-wide chunks to
            # land them on o-partitions (no DRAM bounce - that path
            # yields a NEFF the runtime refuses to load) ----
            ident = cpool.tile([P, P], F32)
            make_identity(nc, ident)
            sqc = cpool.tile([P, OC], F32)
            # one copy releases the nr PSUM bank for m-chunk 1's groups
            nr_sb = sqpool.tile([P, OSH], F32, name="nrb")
            nc.vector.tensor_copy(nr_sb[:], nr[:])
            for oc in range(OC):
                pt = ps_mm.tile([P, P], F32, name="mm")
                nc.tensor.transpose(
                    pt[:], nr_sb[:, oc * P:(oc + 1) * P], ident[:])
                nc.scalar.activation(
                    sqc[:, oc:oc + 1], pt[:, 0:1],
                    mybir.ActivationFunctionType.Sqrt)
            rcp = cpool.tile([P, OC], F32)
            nc.vector.reciprocal(rcp[:], sqc[:])
            scale_col = cpool.tile([P, OC], F32)
            nc.vector.tensor_tensor(scale_col[:], wm_col[:], rcp[:], MULT)

            # ---- m-chunk 0 eviction ----
            for oc in range(OC):
                ot0 = opool.tile([P, MCH], F16, name="ot")
                nc.vector.tensor_scalar(
                    ot0[:], pss0[oc][:],
                    scale_col[:, oc:oc + 1], bias_col[:, oc:oc + 1],
                    MULT, ADD)
                nc.sync.dma_start(outT_v[oc, :, 0:MCH], ot0[:])

            # ---- main matmul: outT[o, m] accumulated over k ----
            for mc in range(1, NM):
                pss = [ps_mm.tile([P, MCH], F32, name="mm")
                       for _ in range(OC)]
                if mc + 2 < NM:
                    j = mc + 2
                    nc.sync.dma_start(
                        xr8[:, :, :, j * MCH:(j + 1) * MCH],
                        xT8.ap()[:, :, :, j * MCH:(j + 1) * MCH])
                xts = []
                for kq in range(3):
                    xt = xpool.tile([P, KO_Q, MCH], F16, name="xt")
                    nc.sync.dma_start(
                        xt[:],
                        xT16.ap()[:, kq * KO_Q:(kq + 1) * KO_Q,
                                  mc * MCH:(mc + 1) * MCH])
                    xts.append(xt)

                # fp16<->fp8 mode switches cost ~1 MM slot each, so
                # alternate the block order per m-chunk: odd m-chunks
                # run DoubleRow FIRST.  Consecutive m-chunks then chain
                # DR|DR and fp16|fp16 across boundaries (m-chunk 0 in
                # the prep loop ends on its DR block -> mc1 starts DR).
                rev = (mc % 2 == 1)

                def emit_dr(oc, first):
                    for kp in range(KP8):
                        nc.tensor.matmul(
                            pss[oc][:],
                            wr8[:, kp, :, oc * P:(oc + 1) * P],
                            xr8[:, kp, :, mc * MCH:(mc + 1) * MCH],
                            start=(first and kp == 0),
                            stop=(not first and kp == KP8 - 1),
                            perf_mode=DR)

                def emit_fp16(oc, kq, first):
                    for k8 in range(KO_Q):
                        nc.tensor.matmul(
                            pss[oc][:],
                            wr16[:, kq * KO_Q + k8, oc * P:(oc + 1) * P],
                            xts[kq][:, k8],
                            start=(first and kq == 0 and k8 == 0),
                            stop=(not first and kq == 2
                                  and k8 == KO_Q - 1))

                def evict(oc):
                    ot = opool.tile([P, MCH], F16)
                    nc.vector.tensor_scalar(
                        ot[:], pss[oc][:],
                        scale_col[:, oc:oc + 1], bias_col[:, oc:oc + 1],
                        MULT, ADD)
                    nc.sync.dma_start(
                        outT_v[oc, :, mc * MCH:(mc + 1) * MCH], ot[:])

                if mc < NM - 1:
                    if rev:
                        for oc in range(OC):
                            emit_dr(oc, first=True)
                        for kq in range(3):
                            for oc in range(OC):
                                emit_fp16(oc, kq, first=False)
                    else:
                        for kq in range(3):
                            for oc in range(OC):
                                emit_fp16(oc, kq, first=True)
                        for oc in range(OC):
                            emit_dr(oc, first=False)
                    for oc in range(OC):
                        evict(oc)
                else:
                    # last m-chunk (odd): DR first chains with mc14's
                    # DR tail; then oc-outer fp16 so each oc closes
                    # early and eviction/store overlap the remaining
                    # groups - shortens the kernel tail
                    for oc in range(OC):
                        emit_dr(oc, first=True)
                    for oc in range(OC):
                        for kq in range(3):
                            emit_fp16(oc, kq, first=False)
                        evict(oc)
    nc.compile()
    return nc


def kernel(x, base_weight, base_bias, weight_m, dora_A, dora_B):
    x = np.asarray(x, dtype=np.float32)
    base_weight = np.asarray(base_weight, dtype=np.float32)
    base_bias = np.asarray(base_bias, dtype=np.float32)
    weight_m = np.asarray(weight_m, dtype=np.float32)
    dora_A = np.asarray(dora_A, dtype=np.float32)
    dora_B = np.asarray(dora_B, dtype=np.float32)

    B, S, _ = x.shape
    assert B * S == M and x.shape[2] == IN

    # x layouts (shared across all cores), pre-scaled by 16:
    #   xT16[p, ko, m] = 16*x[m, ko*128+p]          fp16, ko < 24
    #   xT8[p, kp, t, m] = q8(16*x[m, (24+2kp+t)*128+p])  e4m3
    xs = (x.reshape(M, KO, P) * SX)
    xT16 = np.ascontiguousarray(
        xs[:, :KO16].transpose(2, 1, 0)).astype(np.float16)
    x8part = xs[:, KO16:].reshape(M, KP8, 2, P).transpose(3, 1, 2, 0)
    xT8 = np.clip(np.ascontiguousarray(x8part), -240, 240).astype(
        ml_dtypes.float8_e4m3)
    # A chunks: even k-chunks at partitions 0-15, odd at 32-47
    aT2 = np.zeros((64, KO // 2, P), dtype=np.float16)
    a3 = dora_A.reshape(RANK, KO // 2, 2, P)
    aT2[0:RANK] = a3[:, :, 0]
    aT2[32:32 + RANK] = a3[:, :, 1]

    in_maps = []
    for c in range(NCORES):
        sl = slice(c * OSH, (c + 1) * OSH)
        w_c = base_weight[sl] * SW                              # [OSH, IN]
        wT_c = np.ascontiguousarray(
            w_c.reshape(OSH, KO, P).transpose(2, 1, 0)).astype(np.float16)
        bT_c = np.zeros((64, OSH), dtype=np.float32)
        bT_c[0:RANK] = dora_B[sl].T
        bT_c[32:32 + RANK] = dora_B[sl].T
        wm_c = np.ascontiguousarray(
            (weight_m[sl] / (SX * 32.0)).reshape(OC, P).T)
        bc_c = np.ascontiguousarray(base_bias[sl].reshape(OC, P).T)
        in_maps.append({
            "xT16": xT16,
            "xT8": xT8,
            "wT": wT_c,
            "aT": aT2,
            "bT": bT_c,
            "wm": wm_c,
            "bc": bc_c,
        })

    nc = _build()
    res = run_bass_kernel_spmd(nc, in_maps, core_ids=list(range(NCORES)))

    full = np.empty((OUT, M), dtype=np.float32)
    for c in range(NCORES):
        full[c * OSH:(c + 1) * OSH] = res.results[c]["outT"].astype(
            np.float32)
    return np.ascontiguousarray(full.T).reshape(B, S, OUT)


# revision 54
# speedup vs baseline: 1.0106x; 1.0028x over previous
"""DoRA Linear on 8 Trainium2 NeuronCores (Bass/Tile), fp16 + fp8-DoubleRow.

Reference computation (all fp32):
    new_v   = base_weight + SCALE * dora_B @ dora_A          [OUT, IN]
    scale_o = weight_m / ||new_v||_row                        [OUT]
    out     = x @ (scale_o[:, None] * new_v)^T + base_bias    [B, S, OUT]

Sharding: column-parallel over OUT across 8 cores (OUT/8 = 512 each).
base_weight, dora_B, weight_m, base_bias sharded; x, dora_A replicated.

The main matmul runs in mixed precision to stay under the 2e-2 rel-err
budget while using fp8 DoubleRow (2x PE throughput) where possible:
  - k-chunks 0..23 (3072 of 4096 contraction): fp16 weights and x.
  - k-chunks 24..31: e4m3 fp8, perf_mode=DoubleRow, two 128-k chunks
    per matmul.  Measured error of this 24/8 split: 1.88e-2 (9 fp8
    chunks would be 1.99e-2 - too close to the gate).
Everything is pre-scaled so both parts accumulate in one PSUM group:
weights carry x1024, x carries x16; the /16384 rides in scale_o.

Per-core device program:
  1. Build W'^T = (1024 W + 2048 B@A)^T chunk-by-chunk: PE matmul
     A^T@(2048 B^T) -> PSUM, DVE adds the fp16 1024*W^T chunk, writing
     fp16 wr16 (k<24) or fp8 wr8 pairs (k>=24).
  2. Row norms of the QUANTIZED scaled weights: ACT computes
     sq8 = Square(wr * 2^-5) into e4m3 pairs, PE accumulates
     ones8^T @ sq8 with DoubleRow norm matmuls (one PSUM group, 16 MMs).
     norm matmuls keep M = 128 output partitions (smaller M compiles
     but the runtime refuses the NEFF).
  3. scale_col = (wm/512) / sqrt(nr): PE transpose lands the norms on
     o-partitions, ACT sqrt, DVE reciprocal/mul.  (nr = norm2 * 2^-10.)
  4. Main matmul outT[o, m] = sum_k wr[k, o] * xs[k, m]: 24 fp16 MMs +
     4 fp8 DoubleRow MMs per PSUM group; eviction fuses *scale_o +
     bias_o in one DVE tensor_scalar, output stored fp16 (host upcasts).
Scheduling: ~3.5us of dummy matmuls release the HAM PE clock-gate
(1.2 -> 2.4 GHz) before the real stream; m-chunk 0 matmuls trail the
weight build by one k-pair and norm matmuls by two, so the PE never
waits on the DVE-add -> ACT-square chain.  The whole fp8 x panel
(64KB/partition) is SBUF-resident, streamed in per-m-chunk slices with
+2 lookahead.  All DMAs ride one (sync) queue, ordered critical-first
- a parallel-queue bulk load starves the shared HW DMA engines, and
SWDGE (gpsimd) stores cost a ~7us queue-teardown DRAIN in the tail.
The last m-chunk runs oc-outer so the tail is one eviction + one
small fp16 store.
Host: layout transposes + dtype casts in numpy, final gather/transpose.
"""

import numpy as np
import ml_dtypes

import concourse.mybir as mybir
import concourse.tile as tile
from concourse import bacc
from concourse.bass_utils import run_bass_kernel_spmd
from concourse.masks import make_identity

OUT, IN, RANK = 4096, 4096, 16
SCALE = 2.0
NCORES = 8
OSH = OUT // NCORES          # 512 out features per core
P = 128
KO = IN // P                 # 32 k-chunks
KO16 = 24                    # fp16 k-chunks
KP8 = (KO - KO16) // 2       # 4 fp8 k-pairs (DoubleRow)
KQ = 4                       # k-quarters of 8 chunks (3 fp16 + 1 fp8)
KO_Q = 8
M = 4 * 2048                 # 8192 tokens
MCH = 512                    # tokens per x tile
NM = M // MCH                # 16 m-chunks
OC = OSH // P                # 4 o-chunks of 128
SW = 1024.0                  # weight pre-scale (host)
SX = 16.0                    # x pre-scale (host)
SQS = 2.0 ** -5              # ACT scale for squares: sq = wr^2 * 2^-10

F32 = mybir.dt.float32
F16 = mybir.dt.float16
F8 = mybir.dt.float8e4
DR = mybir.MatmulPerfMode.DoubleRow
ADD = mybir.AluOpType.add
MULT = mybir.AluOpType.mult


def _build():
    nc = bacc.Bacc(None, target_bir_lowering=False)
    xT16 = nc.dram_tensor("xT16", [P, KO16, M], F16, kind="ExternalInput")
    xT8 = nc.dram_tensor("xT8", [P, KP8, 2, M], F8, kind="ExternalInput")
    wT = nc.dram_tensor("wT", [P, KO, OSH], F16, kind="ExternalInput")
    # A chunks for even k at partitions 0-15, odd k at 32-47; B^T
    # duplicated to match: the two B@A matmuls of a k-pair run
    # CONCURRENTLY in disjoint 32-row groups of the PE array
    aT = nc.dram_tensor("aT", [2 * 32, KO // 2, P], F16,
                        kind="ExternalInput")
    bT = nc.dram_tensor("bT", [2 * 32, OSH], F32, kind="ExternalInput")
    wm = nc.dram_tensor("wm", [P, OC], F32, kind="ExternalInput")
    bc = nc.dram_tensor("bc", [P, OC], F32, kind="ExternalInput")
    outT = nc.dram_tensor("outT", [OSH, M], F16, kind="ExternalOutput")
    outT_v = outT.ap().rearrange("(oc p) m -> oc p m", p=P)

    with tile.TileContext(nc) as tc:
        with (
            tc.tile_pool(name="wr", bufs=1) as wrpool,
            tc.tile_pool(name="const", bufs=1) as cpool,
            tc.tile_pool(name="wv", bufs=2) as wvpool,
            tc.tile_pool(name="sq", bufs=5) as sqpool,
            tc.tile_pool(name="xs", bufs=6) as xpool,
            tc.tile_pool(name="os", bufs=4) as opool,
            tc.tile_pool(name="ps_mm", bufs=8, space="PSUM") as ps_mm,
        ):
            # ---- critical-first loads (single sync DMA queue) ----
            bt_f = cpool.tile([2 * 32, OSH], F32)
            nc.sync.dma_start(bt_f[:], bT.ap())
            at_s = cpool.tile([2 * 32, KO // 2, P], F16)
            nc.sync.dma_start(at_s[:], aT.ap())
            # the whole fp8 x panel is only 64KB/partition - keep it
            # SBUF-resident so no DoubleRow matmul ever waits on a DMA
            # gate.  Loads ride the sync queue AFTER the prep-critical
            # pushes (a parallel-queue load hogs the shared HW DMA
            # engines and starves the prep stream for ~24us).
            xr8 = cpool.tile([P, KP8, 2, M], F8)
            ones_f = cpool.tile([P, 2 * P], F32)
            nc.any.memset(ones_f[:], 1.0)
            # DVE order matters: ones8 first (no DMA dep) so warm-up
            # matmuls can issue while bt_f is still in flight
            ones8 = cpool.tile([P, 2, P], F8)
            nc.vector.tensor_copy(ones8[:], ones_f[:])
            bt2 = cpool.tile([2 * 32, OSH], F16)
            nc.vector.tensor_scalar_mul(bt2[:], bt_f[:], SCALE * SW)
            wm_col = cpool.tile([P, OC], F32)
            bias_col = cpool.tile([P, OC], F32)

            # ---- weight prep + m-chunk 0, interleaved per k-pair:
            # wr[:, ko] = 1024*W^T chunk + (2048 B A)^T chunk ----
            wr16 = wrpool.tile([P, KO16, OSH], F16)
            wr8 = wrpool.tile([P, KP8, 2, OSH], F8)
            nr = ps_mm.tile([P, OSH], F32, name="mm")
            # HAM warm-up: ~3.5us of dummy matmuls on ones8 so the PE
            # clock-gate releases (1.2 -> 2.4 GHz) before the real
            # instruction stream begins
            for _ in range(33):
                nc.tensor.matmul(nr[:, 0:P], ones8[:, 0], ones8[:, 0],
                                 start=True, stop=True)
            pss0 = [ps_mm.tile([P, MCH], F32, name="mm") for _ in range(OC)]
            xq_tiles = {}

            def emit_mc0(pair, half=None):
                if pair < 0:
                    return
                if pair < KO16 // 2:
                    kq = pair // (KO_Q // 2)
                    ts = (0, 1) if half is None else (half,)
                    for t in ts:
                        ko = pair * 2 + t
                        k8 = ko - kq * KO_Q
                        if kq == 0:
                            xtf_, xtr_ = xq_tiles[0]
                            xsrc = xtf_[:, k8] if k8 < 2 else xtr_[:, k8 - 2]
                        else:
                            xsrc = xq_tiles[kq][:, k8]
                        for oc in range(OC):
                            nc.tensor.matmul(
                                pss0[oc][:],
                                wr16[:, ko, oc * P:(oc + 1) * P],
                                xsrc,
                                start=(ko == 0), stop=False)
                else:
                    if half == 1:
                        return
                    jp8 = pair - KO16 // 2
                    for oc in range(OC):
                        nc.tensor.matmul(
                            pss0[oc][:],
                            wr8[:, jp8, :, oc * P:(oc + 1) * P],
                            xr8[:, jp8, :, 0:MCH],
                            start=False, stop=(jp8 == KP8 - 1),
                            perf_mode=DR)

            pend_sq = []
            for kq in range(KQ):
                # fine-grained tiles around the critical path: DMA
                # completion gates are whole-tile, so the first chunks'
                # weights/x get their own small tiles
                wva = wvpool.tile([P, 4, OSH], F16, name="wva")
                wvb = wvpool.tile([P, 4, OSH], F16, name="wvb")
                q0 = kq * KO_Q
                nc.sync.dma_start(wva[:], wT.ap()[:, q0:q0 + 4])
                if kq == 0:
                    # single-use tiles -> cpool (bufs=1), keeps the
                    # xpool ring small enough for SBUF
                    xtf = cpool.tile([P, 2, MCH], F16)
                    nc.sync.dma_start(xtf[:], xT16.ap()[:, 0:2, 0:MCH])
                    xt0 = cpool.tile([P, 6, MCH], F16)
                    nc.sync.dma_start(xt0[:], xT16.ap()[:, 2:KO_Q, 0:MCH])
                    xq_tiles[0] = (xtf, xt0)
                elif kq < 3:
                    xt0 = xpool.tile([P, KO_Q, MCH], F16, name="xt")
                    nc.sync.dma_start(
                        xt0[:], xT16.ap()[:, q0:q0 + KO_Q, 0:MCH])
                    xq_tiles[kq] = xt0
                nc.sync.dma_start(wvb[:], wT.ap()[:, q0 + 4:q0 + KO_Q])
                if kq == 1:
                    # m-chunk 0 slice of the fp8 x panel (needed ~45us)
                    nc.sync.dma_start(xr8[:, :, :, 0:MCH],
                                      xT8.ap()[:, :, :, 0:MCH])
                if kq == 3:
                    # small, needed only at scale_col time
                    nc.sync.dma_start(wm_col[:], wm.ap())
                    nc.sync.dma_start(bias_col[:], bc.ap())

                for jp in range(KO_Q // 2):
                    sq8 = sqpool.tile([P, 2, OSH], F8, name="sq8")
                    pair = kq * (KO_Q // 2) + jp
                    bas = [ps_mm.tile([P, OSH], F32, name="mm")
                           for _ in range(2)]
                    for t in range(2):
                        nc.tensor.matmul(
                            bas[t][:], at_s[32 * t:32 * t + RANK, pair],
                            bt2[32 * t:32 * t + RANK],
                            start=True, stop=True)
                    for t in range(2):
                        k8 = 2 * jp + t
                        ko = kq * KO_Q + k8
                        if ko < KO16:
                            wdst = wr16[:, ko]
                        else:
                            wdst = wr8[:, (ko - KO16) // 2, (ko - KO16) % 2]
                        wsrc = wva[:, k8] if k8 < 4 else wvb[:, k8 - 4]
                        nc.vector.tensor_tensor(wdst, wsrc, bas[t][:], ADD)
                        nc.scalar.activation(
                            sq8[:, t], wdst,
                            mybir.ActivationFunctionType.Square, scale=SQS)
                    # m-chunk 0 matmuls one pair behind the weight
                    # build (never waiting on the DVE adds); norm
                    # matmuls trail TWO pairs so the ba->add->square
                    # chain never paces the PE
                    emit_mc0(pair - 1, half=0)
                    if len(pend_sq) == 2:
                        nc.tensor.matmul(
                            nr[:], ones8[:], pend_sq.pop(0)[:],
                            start=(pair == 2), stop=False, perf_mode=DR)
                    pend_sq.append(sq8)
                    emit_mc0(pair - 1, half=1)
            nc.tensor.matmul(
                nr[:], ones8[:], pend_sq[0][:],
                start=False, stop=False, perf_mode=DR)
            emit_mc0(KO // 2 - 1)
            nc.tensor.matmul(
                nr[:], ones8[:], pend_sq[1][:],
                start=False, stop=True, perf_mode=DR)
            # fp8 x panel slices for m-chunks 1+2 (the rest prefetch
            # inside the main loop with +2 m-chunk lookahead)
            for j in (1, 2):
                nc.sync.dma_start(xr8[:, :, :, j * MCH:(j + 1) * MCH],
                                  xT8.ap()[:, :, :, j * MCH:(j + 1) * MCH])

            # ---- scale_col = (wm/512) / sqrt(nr): every row of nr
            # holds the same 512 norms; PE-transpose 128-wide chunks to
            # land them on o-partitions (no DRAM bounce - that path
            # yields a NEFF the runtime refuses to load) ----
            ident = cpool.tile([P, P], F32)
            make_identity(nc, ident)
            sqc = cpool.tile([P, OC], F32)
            # one copy releases the nr PSUM bank for m-chunk 1's groups
            nr_sb = sqpool.tile([P, OSH], F32, name="nrb")
            nc.vector.tensor_copy(nr_sb[:], nr[:])
            for oc in range(OC):
                pt = ps_mm.tile([P, P], F32, name="mm")
                nc.tensor.transpose(
                    pt[:], nr_sb[:, oc * P:(oc + 1) * P], ident[:])
                nc.scalar.activation(
                    sqc[:, oc:oc + 1], pt[:, 0:1],
                    mybir.ActivationFunctionType.Sqrt)
            rcp = cpool.tile([P, OC], F32)
            nc.vector.reciprocal(rcp[:], sqc[:])
            scale_col = cpool.tile([P, OC], F32)
            nc.vector.tensor_tensor(scale_col[:], wm_col[:], rcp[:], MULT)

            # ---- m-chunk 0 eviction ----
            for oc in range(OC):
                ot0 = opool.tile([P, MCH], F16, name="ot")
                nc.vector.tensor_scalar(
                    ot0[:], pss0[oc][:],
                    scale_col[:, oc:oc + 1], bias_col[:, oc:oc + 1],
                    MULT, ADD)
                nc.sync.dma_start(outT_v[oc, :, 0:MCH], ot0[:])

            # ---- main matmul: outT[o, m] accumulated over k ----
            for mc in range(1, NM):
                pss = [ps_mm.tile([P, MCH], F32, name="mm")
                       for _ in range(OC)]
                if mc + 2 < NM:
                    j = mc + 2
                    nc.sync.dma_start(
                        xr8[:, :, :, j * MCH:(j + 1) * MCH],
                        xT8.ap()[:, :, :, j * MCH:(j + 1) * MCH])
                xts = []
                for kq in range(3):
                    xt = xpool.tile([P, KO_Q, MCH], F16, name="xt")
                    nc.sync.dma_start(
                        xt[:],
                        xT16.ap()[:, kq * KO_Q:(kq + 1) * KO_Q,
                                  mc * MCH:(mc + 1) * MCH])
                    xts.append(xt)

                # fp16<->fp8 mode switches cost ~1 MM slot each, so
                # alternate the block order per m-chunk: odd m-chunks
                # run DoubleRow FIRST.  Consecutive m-chunks then chain
                # DR|DR and fp16|fp16 across boundaries (m-chunk 0 in
                # the prep loop ends on its DR block -> mc1 starts DR).
                rev = (mc % 2 == 1)

                def emit_dr(oc, first):
                    for kp in range(KP8):
                        nc.tensor.matmul(
                            pss[oc][:],
                            wr8[:, kp, :, oc * P:(oc + 1) * P],
                            xr8[:, kp, :, mc * MCH:(mc + 1) * MCH],
                            start=(first and kp == 0),
                            stop=(not first and kp == KP8 - 1),
                            perf_mode=DR)

                def emit_fp16(oc, kq, first):
                    for k8 in range(KO_Q):
                        nc.tensor.matmul(
                            pss[oc][:],
                            wr16[:, kq * KO_Q + k8, oc * P:(oc + 1) * P],
                            xts[kq][:, k8],
                            start=(first and kq == 0 and k8 == 0),
                            stop=(not first and kq == 2
                                  and k8 == KO_Q - 1))

                def evict(oc):
                    ot = opool.tile([P, MCH], F16)
                    nc.vector.tensor_scalar(
                        ot[:], pss[oc][:],
                        scale_col[:, oc:oc + 1], bias_col[:, oc:oc + 1],
                        MULT, ADD)
                    nc.sync.dma_start(
                        outT_v[oc, :, mc * MCH:(mc + 1) * MCH], ot[:])

                if mc < NM - 1:
                    if rev:
                        for oc in range(OC):
                            emit_dr(oc, first=True)
                        for kq in range(3):
                            for oc in range(OC):
                                emit_fp16(oc, kq, first=False)
                    else:
                        for kq in range(3):
                            for oc in range(OC):
                                emit_fp16(oc, kq, first=True)
                        for oc in range(OC):
                            emit_dr(oc, first=False)
                    for oc in range(OC):
                        evict(oc)
                else:
                    # last m-chunk (odd): DR first chains with mc14's
                    # DR tail; then oc-outer fp16 so each oc closes
                    # early and eviction/store overlap the remaining
                    # groups - shortens the kernel tail
                    for oc in range(OC):
                        emit_dr(oc, first=True)
                    for oc in range(OC):
                        for kq in range(3):
                            emit_fp16(oc, kq, first=False)
                        evict(oc)
    nc.compile()
    return nc


def kernel(x, base_weight, base_bias, weight_m, dora_A, dora_B):
    x = np.asarray(x, dtype=np.float32)
    base_weight = np.asarray(base_weight, dtype=np.float32)
    base_bias = np.asarray(base_bias, dtype=np.float32)
    weight_m = np.asarray(weight_m, dtype=np.float32)
    dora_A = np.asarray(dora_A, dtype=np.float32)
    dora_B = np.asarray(dora_B, dtype=np.float32)

    B, S, _ = x.shape
    assert B * S == M and x.shape[2] == IN

    # x layouts (shared across all cores), pre-scaled by 16:
    #   xT16[p, ko, m] = 16*x[m, ko*128+p]          fp16, ko < 24
    #   xT8[p, kp, t, m] = q8(16*x[m, (24+2kp+t)*128+p])  e4m3
    xs = (x.reshape(M, KO, P) * SX)
    xT16 = np.ascontiguousarray(
        xs[:, :KO16].transpose(2, 1, 0)).astype(np.float16)
    x8part = xs[:, KO16:].reshape(M, KP8, 2, P).transpose(3, 1, 2, 0)
    xT8 = np.clip(np.ascontiguousarray(x8part), -240, 240).astype(
        ml_dtypes.float8_e4m3)
    # A chunks: even k-chunks at partitions 0-15, odd at 32-47
    aT2 = np.zeros((64, KO // 2, P), dtype=np.float16)
    a3 = dora_A.reshape(RANK, KO // 2, 2, P)
    aT2[0:RANK] = a3[:, :, 0]
    aT2[32:32 + RANK] = a3[:, :, 1]

    in_maps = []
    for c in range(NCORES):
        sl = slice(c * OSH, (c + 1) * OSH)
        w_c = base_weight[sl] * SW                              # [OSH, IN]
        wT_c = np.ascontiguousarray(
            w_c.reshape(OSH, KO, P).transpose(2, 1, 0)).astype(np.float16)
        bT_c = np.zeros((64, OSH), dtype=np.float32)
        bT_c[0:RANK] = dora_B[sl].T
        bT_c[32:32 + RANK] = dora_B[sl].T
        wm_c = np.ascontiguousarray(
            (weight_m[sl] / (SX * 32.0)).reshape(OC, P).T)
        bc_c = np.ascontiguousarray(base_bias[sl].reshape(OC, P).T)
        in_maps.append({
            "xT16": xT16,
            "xT8": xT8,
            "wT": wT_c,
            "aT": aT2,
            "bT": bT_c,
            "wm": wm_c,
            "bc": bc_c,
        })

    nc = _build()
    res = run_bass_kernel_spmd(nc, in_maps, core_ids=list(range(NCORES)))

    full = np.empty((OUT, M), dtype=np.float32)
    for c in range(NCORES):
        full[c * OSH:(c + 1) * OSH] = res.results[c]["outT"].astype(
            np.float32)
    return np.ascontiguousarray(full.T).reshape(B, S, OUT)
